# revision 63
# baseline (speedup 1.0000x reference)
"""Multi-head attention forward (B=4, T=2048, D=1024, H=16), sharded over
8 Trainium2 NeuronCores.

Sharding: data-parallel over batch (4) x query-halves (2). Core c handles
batch b=c//2 and query rows [hf*TQ, (hf+1)*TQ) with hf=c%2, TQ=T//2. Each
core computes K/V over the full (compacted) sequence for its batch element
(duplicated across the 2 cores of a batch -- cheaper than a cross-core
reduce), so the host-side gather is a pure concatenation.

Key compaction: attention is permutation-invariant over key positions, so
the host picks a key ORDER (a layout permutation of x's rows / the mask)
that puts unmasked keys first, and the kernel only touches the first
NKC = ceil(max_unmasked/128) key tiles. Masked/padding keys still flow
through the same on-device mask bias (exp(-1000+s) == 0 in fp32, exactly
like the reference softmax); dropped tiles are all-masked keys whose
softmax weight is exactly 0. The program is compiled per NKC (cached);
the fixed Bernoulli(0.5) mask gives NKC=9 vs 16 full tiles.

All on-device layouts are chosen so the only transpose is a cheap PE
transpose of the attention output:
  x^T (pre-transposed on host as part of the sharding layout)
    Q^T[dq,t] = W_q[din,dq].T @ x^T[din,t]        (lhsT = W_q as stored)
    K^T[dk,t] = W_k[din,dk].T @ x^T[din,t]
    V[t,dv]   = x^T[din,t].T @ W_v[din,dv]        (natural layout)
  S^T[k,q] = K^T[dh,k].T @ Q^T[dh,q]              (keys on partitions)
  P^T = Exp(0.125*S^T + maskbias)  -- one fused ACT op per (head, ktile);
        maskbias varies along k = the partition dim, so it rides the
        per-partition bias operand. No max-subtraction: scores are
        N(0,1)-scaled so exp never overflows fp32.
  PV with P^T STATIONARY (full 128-deep contraction, 65-cycle matmuls):
    att_q[q, 0:64|Z] = P^T[k,q].T @ [V_h | 1][k, 65], accumulated over kt.
    This is ~2.2x cheaper on PE than the [65, 512] orientation (which
    fills only 65 of 128 output partitions).
  normalize on DVE: zinv = 1/att_q[:, 64] (per-partition scalar), then
    att_sb = att_q[:, 0:64] * zinv broadcast along free (step-0 read).
  att^T via PE transpose ([128 q, 64] -> [64, 128] blocks into [din, t])
  y[t,dc] = att^T[din,t].T @ W_out[din,dc] + b_out (natural layout -> DMA)

PSUM budget (8 banks): st 2x2 (S^T tiles, also time-shared by the V and
out projections as [128,512] halves -- avoids any pool-close barrier),
att_q 1x2, transpose 1x1, kq-filler 1x1.

Scheduling (the PE is the globally binding engine at ~233us busy, the
ACT exp stream is second at ~153us, so every idle PE cycle is wall
time):
  - 24 junk matmuls on a zeroed tile warm the HAM clock gate (PE at
    1.2GHz until ~3.4us of sustained activity) inside the DMA-ramp
    shadow, so the V projection starts at 2.4GHz.
  - DMA order is bandwidth-critical: mask first (a late mask head-of-
    line-blocks V's bias-adds in the strict-FIFO DVE queue), then bv,
    wv, xk in 512-col chunks (~128KB per dma_start spreads the 16
    queues; bigger single DMAs serialize, smaller ones waste the
    descriptor sweet spot), then the prefetched t2=0 K/Q weight blocks,
    xq, and the late-needed bo/wout.
  - exp(kt) on ACT runs while PE does S^T(kt+1) then the 8 PV matmuls
    of kt; K^T/Q^T projection matmul groups are popped between kt steps
    as filler. The S^T stream runs one unit AHEAD across (head-pair,
    q-chunk) boundaries so exp(0) of the next unit never waits out the
    boundary.
  - the first NJP=6 j-tiles of the output projection are accumulated
    into an SBUF scratch (bias folded in) as late-phase filler -- they
    only need ATT[0..5], so they pop once head-pair 6 starts -- leaving
    a j=6,7-only serial tail; y rows 0:TQ/2 additionally drain during
    the last unit's kt loop. y is written bf16 (the fp32 write would
    double the final DMA drain; bf16 costs ~3e-4 of absmax_rel).
"""

import os
import sys
import types
from contextlib import ExitStack

import numpy as np
import ml_dtypes

import concourse.bass as bass
import concourse.tile as tile
import concourse.mybir as mybir
from concourse import bacc, bass_utils

P = 128
DH = 64

F32 = mybir.dt.float32
BF16 = mybir.dt.bfloat16
F32R = mybir.dt.float32r

# Full-problem dims (hardcoded per contract).
FULL_DIMS = dict(B=4, T=2048, D=1024, H=16)

DEFAULT_CFG = dict(
    dt_x=BF16,      # xT / xqT storage (dram + sbuf)
    dt_w=BF16,      # W_qkv streaming blocks
    dt_kv=BF16,     # K^T and V(aug) sbuf storage; must equal dt_p
    dt_q=BF16,      # Q^T sbuf storage
    dt_p=BF16,      # P^T (softmax numerator) sbuf storage
    dt_att=BF16,    # att^T and W_out storage
    use_f32r=True,  # bitcast fp32 matmul operands to float32r (4x faster)
)


def _np_dt(dt):
    return {F32: np.float32, BF16: ml_dtypes.bfloat16}[dt]


def _install_ntff_shim():
    """The agent image's antenv lacks axon_hooks; bass_utils needs it for
    trace=True under axon. Provide it from the boot module."""
    if "antenv.axon_hooks" in sys.modules:
        return
    try:
        from trn_agent_boot.trn_boot import _ntff_profile_via_ctypes
        hook = _ntff_profile_via_ctypes("/opt/axon/libaxon_pjrt.so")
    except Exception:
        hook = None
    mod = types.ModuleType("antenv.axon_hooks")
    mod.get_axon_ntff_profile_hook = lambda: hook
    mod.set_axon_ntff_profile_hook = lambda h: None
    sys.modules["antenv.axon_hooks"] = mod


def _chunks(total, sz):
    out, off = [], 0
    while off < total:
        c = min(sz, total - off)
        out.append((off, c))
        off += c
    return out


def build_nc(dims, cfg, NKC):
    """Build the per-core SPMD program for NKC compacted key tiles."""
    T, D, H = dims["T"], dims["D"], dims["H"]
    assert H * DH == D
    TQ = T // 2           # queries per core
    NDIN = D // P         # contraction tiles for the projections
    NHT = H // 2          # head pairs (2 heads of 64 per 128 partitions)
    TKC = NKC * P         # compacted key positions
    FBV = min(512, D)     # dv-block for V compute
    FBO = min(512, D)     # dc-block for out projection
    D3 = 3 * D

    dt_x, dt_w = cfg["dt_x"], cfg["dt_w"]
    dt_kv, dt_q, dt_p, dt_att = cfg["dt_kv"], cfg["dt_q"], cfg["dt_p"], cfg["dt_att"]
    assert dt_p == dt_kv, "PV matmul needs matching operand dtypes"

    # SBUF headroom fallback for near-unmasked inputs (rare: the mask is
    # Bernoulli(0.5), so NKC ~ T/256; these trims only cost a little overlap)
    big = NKC > 12
    wblk_bufs = 2 if big else 3
    wv_bufs = 1 if big else 2
    pt_bufs = 2 if big else 4
    ob_bufs = 3 if big else 6
    dt_bias = BF16  # bias magnitudes ~0.06; bf16 rounding is ~2e-4 absolute

    def mm(ap):
        if cfg["use_f32r"] and ap.dtype == F32:
            return ap.bitcast(F32R)
        return ap

    nc = bacc.Bacc("TRN2", target_bir_lowering=False, debug=False)

    xkT_d = nc.dram_tensor("xkT", [D, TKC], dt_x, kind="ExternalInput")
    xqT_d = nc.dram_tensor("xqT", [D, TQ], dt_x, kind="ExternalInput")
    wqkv_d = nc.dram_tensor("wqkv", [D, D3], dt_w, kind="ExternalInput")
    wout_d = nc.dram_tensor("wout", [D, D], dt_att, kind="ExternalInput")
    bq_d = nc.dram_tensor("bq", [P, NDIN], F32, kind="ExternalInput")
    bk_d = nc.dram_tensor("bk", [P, NDIN], F32, kind="ExternalInput")
    bv_d = nc.dram_tensor("bv", [P, D], dt_bias, kind="ExternalInput")
    bo_d = nc.dram_tensor("bo", [P, D], dt_bias, kind="ExternalInput")
    maskm_d = nc.dram_tensor("maskm", [P, NKC], F32, kind="ExternalInput")
    ident_d = nc.dram_tensor("ident", [P, P], dt_att, kind="ExternalInput")
    y_d = nc.dram_tensor("y", [TQ, D], BF16, kind="ExternalOutput")

    in_names = ["xkT", "xqT", "wqkv", "wout", "bq", "bk", "bv", "bo",
                "maskm", "ident"]

    # wqkv viewed as [p, din_tile, col] so one DMA grabs a column block
    # across all NDIN din tiles.
    wqkv_v = wqkv_d.ap().rearrange("(j p) n -> p j n", p=P)
    wout_v = wout_d.ap().rearrange("(j p) n -> p j n", p=P)

    EXP = mybir.ActivationFunctionType.Exp

    with tile.TileContext(nc) as tc, ExitStack() as stk:
        misc = stk.enter_context(tc.tile_pool(name="misc", bufs=1))
        pers = stk.enter_context(tc.tile_pool(name="pers", bufs=1))

        # --- small persistent tiles (no DMAs yet: the first ~10us of DMA
        # bandwidth is reserved for the V-projection critical path) --------
        bv_sb = misc.tile([P, D], dt_bias, tag="bv", name="bv_sb")
        mf_sb = misc.tile([P, NKC], F32, tag="mf", name="mf_sb")
        id_sb = misc.tile([P, P], dt_att, tag="ident", name="id_sb")
        # mask first: tiny, and anything DVE-dependent on it must never
        # head-of-line-block the V bias-adds in the strict-FIFO DVE queue
        nc.sync.dma_start(out=mf_sb, in_=maskm_d.ap())

        # --- persistent big tensors ----------------------------------------
        KT = [pers.tile([P, TKC], dt_kv, tag=f"KT{i}", name=f"KT{i}")
              for i in range(NDIN)]
        QT = [pers.tile([P, TQ], dt_q, tag=f"QT{i}", name=f"QT{i}")
              for i in range(NDIN)]
        VA = [pers.tile([P, H * (DH + 1)], dt_kv, tag=f"VA{i}", name=f"VA{i}")
              for i in range(NKC)]
        ATT = [pers.tile([P, TQ], dt_att, tag=f"ATT{i}", name=f"ATT{i}")
               for i in range(NDIN)]

        # ones columns of the augmented V
        for kt in range(NKC):
            va_v = VA[kt].rearrange("p (h c) -> p h c", c=DH + 1)
            nc.vector.memset(va_v[:, :, DH:DH + 1], 1.0)

        # ========== Phase 1+2: projections interleaved with attention ======
        # V is computed first (every PV needs all of it). The K^T/Q^T
        # projection matmul groups are then fed into the attention emission
        # as filler work: phase 2 is ACT(exp)-throughput-bound and the PE
        # queue is in-order, so projection MMs slotted between attention MMs
        # keep the PE busy (and the HAM clock-gate warm) while ACT catches
        # up. Head h needs K^T/Q^T tile h//2, so the filler queue is ordered
        # by head-pair and drained ahead of each head's first matmul.
        with tc.tile_pool(name="ph1", bufs=1) as ph1, \
             tc.tile_pool(name="wstr", bufs=1) as wstr, \
             tc.tile_pool(name="ph2", bufs=1) as ph2, \
             tc.tile_pool(name="wvp", bufs=1) as wvp, \
             tc.tile_pool(name="stps", bufs=1, space="PSUM") as stps, \
             tc.tile_pool(name="kqps", bufs=1, space="PSUM") as kqps:

            # st tiles: [128, 1024] f32 (2 banks, 2 bufs). Attention S^T
            # uses the full width; the V projection and out projection use
            # [128, 512] halves of the same tag so they pipeline into/out of
            # attention with no pool barrier and no extra banks.
            def st_tile(nm):
                return stps.tile([P, 1024], F32, tag="st", bufs=2, name=nm)

            # the first V matmul group needs wv(0) and the xk tiles in
            # j-order; split the wv DMA per din-tile so it spreads across
            # DMA queues instead of serializing ~1MB on one queue, and feed
            # the xk columns in ascending-size chunks so group (dv2=0,kt=0)
            # is gated on only ~1.25MB of traffic.
            hpb = FBV // DH  # heads per dv block
            if True:
                # PE warm-up in the DMA shadow: the HAM clock gate holds the
                # PE at 1.2GHz until ~3.4us of sustained activity; junk
                # matmuls on a zeroed tile (no DMA deps) warm it for free.
                warm = misc.tile([P, 512], BF16, tag="warm", name="warm")
                nc.vector.memset(warm, 0.0)
                for wi in range(24):
                    wps = stps.tile([P, 1024], F32, tag="st", bufs=2,
                                    name=f"warm{wi}")
                    nc.tensor.matmul(wps[:, 0:512], warm[:, 0:P], warm,
                                     start=True, stop=True)

                # bv first: only 256KB, and the V bias-adds must never wait
                # on it (a late bv head-of-line-blocks the st psum ring).
                nc.sync.dma_start(out=bv_sb, in_=bv_d.ap())

                wvs = []
                for dv2 in range(D // FBV):
                    wvs.append(wvp.tile([P, NDIN, FBV], dt_w, tag="wv",
                                        bufs=wv_bufs, name=f"wv{dv2}"))

                def wv_dma(dv2):
                    for j in range(NDIN):
                        nc.sync.dma_start(
                            out=wvs[dv2][:, j, :],
                            in_=wqkv_v[:, j, 2 * D + dv2 * FBV:
                                       2 * D + (dv2 + 1) * FBV])

                xks = [ph1.tile([P, TKC], dt_x, tag=f"xk{j}", name=f"xk{j}")
                       for j in range(NDIN)]

                def xk_dma(off, csz):
                    for j in range(NDIN):
                        nc.sync.dma_start(
                            out=xks[j][:, off:off + csz],
                            in_=xkT_d.ap()[j * P:(j + 1) * P, off:off + csz])

                wv_dma(0)
                for oc in _chunks(TKC, 512):
                    xk_dma(*oc)
                wv_dma(1)
                nc.sync.dma_start(out=id_sb, in_=ident_d.ap())
                # prefetch the first K/Q weight blocks so the t2=0 filler
                # groups never wait on their DMAs behind the xq stream
                wbk0_pre = wstr.tile([P, NDIN, P], dt_w, tag="wblk",
                                     bufs=wblk_bufs, name="wbk0")
                nc.sync.dma_start(out=wbk0_pre, in_=wqkv_v[:, :, D:D + P])
                wbq0_pre = wstr.tile([P, NDIN, P], dt_w, tag="wblk",
                                     bufs=wblk_bufs, name="wbq0")
                nc.sync.dma_start(out=wbq0_pre, in_=wqkv_v[:, :, 0:P])
                def v_group(dv2, kt, ps):
                    psh = ps[:, 0:FBV]
                    wv = wvs[dv2]
                    for j in range(NDIN):
                        nc.tensor.matmul(
                            psh, mm(xks[j][:, kt * P:(kt + 1) * P]),
                            mm(wv[:, j, :]),
                            start=(j == 0), stop=(j == NDIN - 1))
                    va_v = VA[kt].rearrange("p (h c) -> p h c", c=DH + 1)
                    nc.vector.tensor_add(
                        va_v[:, dv2 * hpb:(dv2 + 1) * hpb, 0:DH],
                        psh.rearrange("p (h c) -> p h c", c=DH),
                        bv_sb[:, dv2 * FBV:(dv2 + 1) * FBV]
                        .rearrange("p (h c) -> p h c", c=DH))

                for dv2 in range(D // FBV):
                    for kt in range(NKC):
                        v_group(dv2, kt, st_tile(f"vps{dv2}_{kt}"))

            # mask bias prep: emitted only now so these DVE ops sit BEHIND
            # the V bias-adds in the strict-FIFO DVE queue.
            m1_sb = misc.tile([P, NKC], F32, tag="m1", name="m1_sb")
            nc.vector.tensor_scalar_add(m1_sb, mf_sb, -1.0)
            maskadd = misc.tile([P, NKC], F32, tag="maskadd",
                                name="maskadd")
            nc.vector.tensor_scalar_mul(maskadd, m1_sb, 1000.0)

            # loads not needed until the K/Q fillers and the projection --
            # emitted after V so they don't delay the first V matmuls.
            bq_sb = misc.tile([P, NDIN], F32, tag="bq", name="bq_sb")
            nc.sync.dma_start(out=bq_sb, in_=bq_d.ap())
            bk_sb = misc.tile([P, NDIN], F32, tag="bk", name="bk_sb")
            nc.sync.dma_start(out=bk_sb, in_=bk_d.ap())
            xqs = []
            for j in range(NDIN):
                xq = ph1.tile([P, TQ], dt_x, tag=f"xq{j}", name=f"xq{j}")
                nc.sync.dma_start(out=xq, in_=xqT_d.ap()[j * P:(j + 1) * P, :])
                xqs.append(xq)
            bo_sb = misc.tile([P, D], dt_bias, tag="bo", name="bo_sb")
            nc.sync.dma_start(out=bo_sb, in_=bo_d.ap())
            wout_sb = []
            for j in range(NDIN):
                wo = ph2.tile([P, D], dt_att, tag=f"wo{j}", name=f"wo{j}")
                nc.sync.dma_start(out=wo, in_=wout_v[:, j, :])
                wout_sb.append(wo)

            # --- K^T / Q^T filler work queue, ordered by head-pair -----
            def kq_dma(col0, nm):
                wb = wstr.tile([P, NDIN, P], dt_w, tag="wblk",
                               bufs=wblk_bufs, name=nm)
                nc.sync.dma_start(
                    out=wb, in_=wqkv_v[:, :, col0:col0 + P])
                return wb

            def kq_group(wb, xs, dst, bias, off, csz, nm):
                ps = kqps.tile([P, 512], F32, tag="kps", bufs=1, name=nm)
                for j in range(NDIN):
                    nc.tensor.matmul(
                        ps[:, :csz], mm(wb[:, j, :]),
                        mm(xs[j][:, off:off + csz]),
                        start=(j == 0), stop=(j == NDIN - 1))
                nc.vector.tensor_scalar_add(
                    dst[:, off:off + csz], ps[:, :csz], bias)

            work = []  # (hp, closure)
            for t2 in range(NDIN):
                wbk_hold, wbq_hold = {}, {}
                if t2 == 0:
                    wbk_hold["wb"] = wbk0_pre
                    wbq_hold["wb"] = wbq0_pre

                def mk_dma(hold, col0, nm):
                    def run():
                        hold["wb"] = kq_dma(col0, nm)
                    return run

                def mk_grp(hold, xs, dst, bias, off, csz, nm):
                    def run():
                        kq_group(hold["wb"], xs, dst, bias, off, csz, nm)
                    return run

                if t2 > 0:
                    work.append((t2, mk_dma(wbk_hold, D + t2 * P,
                                            f"wbk{t2}")))
                for off, csz in _chunks(TKC, 512):
                    work.append((t2, mk_grp(
                        wbk_hold, xks, KT[t2], bk_sb[:, t2:t2 + 1],
                        off, csz, f"kps{t2}_{off}")))
                if t2 > 0:
                    work.append((t2, mk_dma(wbq_hold, t2 * P, f"wbq{t2}")))
                for off, csz in _chunks(TQ, 512):
                    work.append((t2, mk_grp(
                        wbq_hold, xqs, QT[t2], bq_sb[:, t2:t2 + 1],
                        off, csz, f"qps{t2}_{off}")))

            # Out-projection split: j=0..NJP-1 accumulate into an SBUF
            # scratch (with the output bias folded in) as filler work
            # during the LAST head pair -- exactly where the K/Q filler
            # queue runs dry and the PE otherwise idles on exp -- leaving
            # only j=NJP..7 for the serial tail.
            NJP = 0 if big else 6
            if NJP:
                yacc = ph2.tile([P, TQ // P, D], F32, tag="yacc",
                                name="yacc")
            part_work = []

            def mk_part(tb, dc):
                def run():
                    ps = kqps.tile([P, 512], F32, tag="kps", bufs=1,
                                   name=f"pp{tb}_{dc}")
                    for j in range(NJP):
                        nc.tensor.matmul(
                            ps[:, :FBO],
                            mm(ATT[j][:, tb * P:(tb + 1) * P]),
                            mm(wout_sb[j][:, dc * FBO:(dc + 1) * FBO]),
                            start=(j == 0), stop=(j == NJP - 1))
                    nc.vector.tensor_add(
                        yacc[:, tb, dc * FBO:(dc + 1) * FBO], ps[:, :FBO],
                        bo_sb[:, dc * FBO:(dc + 1) * FBO])
                return run

            if NJP:
                for tb in range(TQ // P):
                    for dc in range(D // FBO):
                        part_work.append(mk_part(tb, dc))

            widx = [0]
            pidx = [0]

            def drain_kq(hp_needed):
                while widx[0] < len(work) and \
                        work[widx[0]][0] <= hp_needed:
                    work[widx[0]][1]()
                    widx[0] += 1

            def pop_kq(n=1, parts=False):
                for _ in range(n):
                    if widx[0] < len(work):
                        work[widx[0]][1]()
                        widx[0] += 1
                    elif parts and pidx[0] < len(part_work):
                        part_work[pidx[0]]()
                        pidx[0] += 1

            # --- attention ---------------------------------------------
            # Head PAIRS share one [128, 2*512] score tile: head 0's
            # q-chunk in cols [0,512), head 1's in [512,1024) (separate
            # psum banks). One TQ-wide exp covers both (same per-partition
            # mask bias). PV runs with P^T stationary: per (s2, 128-wide
            # q subtile), out_q[128 q, 65] += P^T.T @ [V_h|1], full-128
            # contraction, 65-cycle matmuls, accumulated over kt.
            slot = [0]
            qhs = _chunks(TQ, 512)
            STW = 512
            NSUB = STW // P  # 128-wide q subtiles per q-chunk
            fin_pend = [None]  # deferred normalize+transpose closure

            units = [(hp, qi, off, qcsz)
                     for hp in range(NHT)
                     for qi, (off, qcsz) in enumerate(qhs)]

            def mk_st(_hp, _qi, _off, _qcsz):
                def st_mm(kt):
                    stt = st_tile(f"st{_hp}_{_qi}_{kt}")
                    for s2 in range(2):
                        b2 = s2 * DH
                        nc.tensor.matmul(
                            stt[:, s2 * STW:s2 * STW + _qcsz],
                            mm(KT[_hp][b2:b2 + DH,
                                       kt * P:(kt + 1) * P]),
                            mm(QT[_hp][b2:b2 + DH,
                                       _off:_off + _qcsz]),
                            start=True, stop=True)
                    return stt
                return st_mm

            def tail_tb(tb):
                ob = ph2.tile([P, D], BF16, tag="ob", bufs=ob_bufs,
                              name=f"ob{tb}")
                for dc in range(D // FBO):
                    ps = st_tile(f"ops{tb}_{dc}")
                    psh = ps[:, 0:FBO]
                    for j in range(NJP, NDIN):
                        nc.tensor.matmul(
                            psh,
                            mm(ATT[j][:, tb * P:(tb + 1) * P]),
                            mm(wout_sb[j][:, dc * FBO:(dc + 1) * FBO]),
                            start=(j == NJP), stop=(j == NDIN - 1))
                    if NJP:
                        nc.vector.tensor_add(
                            ob[:, dc * FBO:(dc + 1) * FBO], psh,
                            yacc[:, tb, dc * FBO:(dc + 1) * FBO])
                    else:
                        nc.vector.tensor_add(
                            ob[:, dc * FBO:(dc + 1) * FBO], psh,
                            bo_sb[:, dc * FBO:(dc + 1) * FBO])
                for dq in range(2):
                    nc.sync.dma_start(
                        out=y_d.ap()[tb * P:(tb + 1) * P,
                                     dq * (D // 2):(dq + 1) * (D // 2)],
                        in_=ob[:, dq * (D // 2):(dq + 1) * (D // 2)])

            stfns = [mk_st(*u) for u in units]
            st_ahead = [None]  # next unit's st(0), emitted one unit early
            tail_done = set()

            for ui, (hp, qi, off, qcsz) in enumerate(units):
                if qi == 0:
                    drain_kq(hp)
                nsub = (qcsz + P - 1) // P

                # att_q psum: per s2 one [128, nsub, 65] tile (1 bank)
                aqs = [stps.tile([P, NSUB, DH + 1], F32, tag="attq",
                                 bufs=2, name=f"aq{hp}_{qi}_{s2}")
                       for s2 in range(2)]
                if st_ahead[0] is not None:
                    stt = st_ahead[0]
                    st_ahead[0] = None
                else:
                    stt = stfns[ui](0)
                for kt in range(NKC):
                    pt = ph2.tile([P, 2 * STW], dt_p, tag="pt",
                                  bufs=pt_bufs,
                                  name=f"pt{hp}_{qi}_{kt}")
                    if qcsz == STW:
                        nc.scalar.activation(
                            out=pt, in_=stt, func=EXP,
                            bias=maskadd[:, kt:kt + 1], scale=0.125)
                    else:
                        for s2 in range(2):
                            nc.scalar.activation(
                                out=pt[:, s2 * STW:s2 * STW + qcsz],
                                in_=stt[:, s2 * STW:s2 * STW + qcsz],
                                func=EXP,
                                bias=maskadd[:, kt:kt + 1],
                                scale=0.125)
                    if kt + 1 < NKC:
                        stt = stfns[ui](kt + 1)
                    elif ui + 1 < len(units):
                        # S-stream runs one unit AHEAD: the next unit's
                        # st(0) is emitted before this unit's last PVs so
                        # its exp(0) never waits out the unit boundary.
                        if units[ui + 1][1] == 0:
                            drain_kq(units[ui + 1][0])
                        st_ahead[0] = stfns[ui + 1](0)
                    # previous unit's finalize lands here: its PE
                    # transposes slot into the wait for exp(0).
                    if kt == 0 and fin_pend[0] is not None:
                        fin_pend[0]()
                        fin_pend[0] = None
                    # start=True only on the tile's FIRST write: the
                    # hardware zero-region is the whole 2KB bank, so a
                    # per-qsub start would wipe earlier qsubs' kt=0
                    # results. Later qsubs' first writes land on
                    # still-pending bytes and overwrite correctly.
                    for s2 in range(2):
                        h2 = 2 * hp + s2
                        for sq in range(nsub):
                            scs = min(P, qcsz - sq * P)
                            nc.tensor.matmul(
                                aqs[s2][0:scs, sq, :],
                                mm(pt[:, s2 * STW + sq * P:
                                      s2 * STW + sq * P + scs]),
                                mm(VA[kt][:, h2 * (DH + 1):
                                          (h2 + 1) * (DH + 1)]),
                                start=(kt == 0 and sq == 0),
                                stop=(kt == NKC - 1 and sq == nsub - 1),
                                skip_group_check=True)
                    slot[0] += 1
                    # partials read ATT[0..NJP-1]; ATT[NJP-1] is written by
                    # fin(NJP-1, qi=1), emitted at unit (NJP, 0) kt==0 -- so
                    # partial pops are safe only from (NJP, 0) kt>=1 on.
                    psafe = ui > 2 * NJP or (ui == 2 * NJP and kt >= 1)
                    if slot[0] % 3 == 0:
                        pop_kq(1, parts=psafe)
                    # during the last unit, rows 0:TQ/2 of y depend only on
                    # fin(last hp, qi=0) and the finished partials -- emit
                    # their tail groups now so their DMAs drain early.
                    if (ui == len(units) - 1 and kt >= 1
                            and pidx[0] >= len(part_work)
                            and widx[0] >= len(work)
                            and len(tail_done) < TQ // (2 * P)):
                        tb = len(tail_done)
                        tail_tb(tb)
                        tail_done.add(tb)

                # Deferred finalize: 1/Z on DVE (Z = ones-column 64 of
                # att_q; per-partition scalars in this orientation),
                # normalize+downcast with a step-0 free-dim broadcast
                # read of zinv, then PE-transpose the [128 q, 64]
                # blocks into ATT's [din, t] layout. Runs after the
                # next unit's first exp is queued so the PE never
                # stalls on the DVE chain.
                def fin(_hp=hp, _qi=qi, _off=off, _qcsz=qcsz,
                        _aqs=aqs, _nsub=nsub):
                    zinv = ph2.tile([P, 2, NSUB], F32, tag="zinv",
                                    bufs=2, name=f"zi{_hp}_{_qi}")
                    asb = ph2.tile([P, NSUB, 2, DH], dt_att, tag="asb",
                                   bufs=2, name=f"as{_hp}_{_qi}")
                    tp = stps.tile([P, STW], dt_att, tag="tp",
                                   bufs=1, name=f"tp{_hp}_{_qi}")
                    for s2 in range(2):
                        nc.vector.reciprocal(
                            zinv[:, s2, 0:_nsub],
                            _aqs[s2][:, 0:_nsub, DH])
                        zbc = bass.AP(
                            tensor=zinv.tensor,
                            offset=zinv.offset + s2 * NSUB,
                            ap=[zinv.ap[0], [1, _nsub], [0, DH]])
                        nc.vector.tensor_mul(
                            asb[:, 0:_nsub, s2, :],
                            _aqs[s2][:, 0:_nsub, 0:DH], zbc)
                    # one transpose per q-subtile covers BOTH heads: the
                    # strided lhsT [128 q, (2 s2, 64 dh)] transposes to
                    # [128 din, q] -- exactly ATT's layout.
                    for sq in range(_nsub):
                        scs = min(P, _qcsz - sq * P)
                        nc.tensor.transpose(
                            tp[:, sq * P:sq * P + scs],
                            asb[0:scs, sq, :, :],
                            id_sb[0:scs, 0:scs])
                    nc.vector.tensor_copy(
                        ATT[_hp][:, _off:_off + _qcsz],
                        tp[:, 0:_qcsz])
                fin_pend[0] = fin
            fin_pend[0]()
            fin_pend[0] = None
            pop_kq(len(work))

            # ========= Phase 3: output projection tail =================
            # Only j=NJP..7 remain (the rest was absorbed as phase-2
            # filler); rides the st psum tag so no pool-close barrier
            # separates attention from the projection. One full-row ob
            # tile per tb gives the y DMA 2KB-per-partition descriptors.
            # drain leftover partials PROGRESSIVELY: tail_tb(tb) needs
            # only partials 2tb..2tb+1, so early tb tiles (whose partials
            # finished long ago) ship immediately instead of waiting out a
            # serial blob of the last few high-tb partials.
            for tb in range(TQ // P):
                while pidx[0] < min(2 * (tb + 1), len(part_work)):
                    part_work[pidx[0]]()
                    pidx[0] += 1
                if tb not in tail_done:
                    tail_tb(tb)

    nc.compile()
    return nc, in_names


def shard_inputs(dims, cfg, NKC, x, mask, W_qkv, b_qkv, W_out, b_out):
    """Host-side sharding: slices, layout transposes/permutation, bias
    tiling. The key permutation puts unmasked keys first (padding keeps
    mask=0 so the device-side bias kills it)."""
    B, T, D = dims["B"], dims["T"], dims["D"]
    TQ = T // 2
    NDIN = D // P
    TKC = NKC * P
    npx = _np_dt(cfg["dt_x"])
    npw = _np_dt(cfg["dt_w"])
    npa = _np_dt(cfg["dt_att"])

    x = np.asarray(x)
    mask = np.asarray(mask)
    W_qkv = np.asarray(W_qkv)
    b_qkv = np.asarray(b_qkv)
    W_out = np.asarray(W_out)
    b_out = np.asarray(b_out)

    wqkv_c = np.ascontiguousarray(W_qkv.astype(npw))
    wout_c = np.ascontiguousarray(W_out.astype(npa))
    bq = np.ascontiguousarray(b_qkv[:D].reshape(NDIN, P).T.astype(np.float32))
    bk = np.ascontiguousarray(
        b_qkv[D:2 * D].reshape(NDIN, P).T.astype(np.float32))
    np_bias = _np_dt(BF16)
    bv = np.ascontiguousarray(
        np.broadcast_to(b_qkv[2 * D:], (P, D)).astype(np_bias))
    bo = np.ascontiguousarray(
        np.broadcast_to(b_out, (P, D)).astype(np_bias))
    ident = np.ascontiguousarray(np.eye(P, dtype=npa))

    in_maps = []
    percore = {}
    for b in range(B):
        mb = mask[b, 0, 0]
        idx_on = np.nonzero(mb == 1)[0]
        perm = np.zeros(TKC, dtype=np.int64)  # pad with key 0 (masked off)
        perm[:len(idx_on)] = idx_on
        mc = np.zeros(TKC, dtype=np.float32)
        mc[:len(idx_on)] = 1.0
        xkT = np.ascontiguousarray(x[b][perm].T.astype(npx))
        maskm = np.ascontiguousarray(mc.reshape(NKC, P).T)
        percore[b] = (xkT, maskm)

    for c in range(2 * B):
        b, hf = c // 2, c % 2
        xkT, maskm = percore[b]
        xqT = np.ascontiguousarray(
            x[b, hf * TQ:(hf + 1) * TQ, :].T.astype(npx))
        in_maps.append(dict(
            xkT=xkT, xqT=xqT, wqkv=wqkv_c, wout=wout_c,
            bq=bq, bk=bk, bv=bv, bo=bo, maskm=maskm, ident=ident))
    return in_maps


_CACHE = {}
LAST_EXEC_NS = None


def kernel(x, mask, W_qkv, b_qkv, W_out, b_out):
    global LAST_EXEC_NS
    dims = FULL_DIMS
    cfg = DEFAULT_CFG
    _install_ntff_shim()

    mask = np.asarray(mask)
    counts = mask.reshape(dims["B"], -1).sum(1)
    NKC = max(1, int(np.ceil(counts.max() / P)))
    NKC = min(NKC, dims["T"] // P)

    if NKC not in _CACHE:
        _CACHE[NKC] = build_nc(dims, cfg, NKC)
    nc, _ = _CACHE[NKC]

    in_maps = shard_inputs(dims, cfg, NKC, x, mask, W_qkv, b_qkv,
                           W_out, b_out)
    trace = bool(os.environ.get("KERNEL_TRACE"))
    res = bass_utils.run_bass_kernel_spmd(
        nc, in_maps, core_ids=list(range(8)), trace=trace,
        tmpdir=os.environ.get("KERNEL_TRACE_DIR") or None)
    LAST_EXEC_NS = res.exec_time_ns

    B, T, D = dims["B"], dims["T"], dims["D"]
    TQ = T // 2
    out = np.empty((B, T, D), dtype=np.float32)
    for c in range(2 * B):
        b, hf = c // 2, c % 2
        out[b, hf * TQ:(hf + 1) * TQ, :] = np.asarray(
            res.results[c]["y"], dtype=np.float32)
    return out


# revision 64
# speedup vs baseline: 1.0109x; 1.0109x over previous
"""Multi-head attention forward (B=4, T=2048, D=1024, H=16), sharded over
8 Trainium2 NeuronCores.

Sharding: data-parallel over batch (4) x query-halves (2). Core c handles
batch b=c//2 and query rows [hf*TQ, (hf+1)*TQ) with hf=c%2, TQ=T//2. Each
core computes K/V over the full (compacted) sequence for its batch element
(duplicated across the 2 cores of a batch -- cheaper than a cross-core
reduce), so the host-side gather is a pure concatenation.

Key compaction: attention is permutation-invariant over key positions, so
the host picks a key ORDER (a layout permutation of x's rows / the mask)
that puts unmasked keys first, and the kernel only touches the first
NKC = ceil(max_unmasked/128) key tiles. Masked/padding keys still flow
through the same on-device mask bias (exp(-1000+s) == 0 in fp32, exactly
like the reference softmax); dropped tiles are all-masked keys whose
softmax weight is exactly 0. The program is compiled per NKC (cached);
the fixed Bernoulli(0.5) mask gives NKC=9 vs 16 full tiles.

All on-device layouts are chosen so the only transpose is a cheap PE
transpose of the attention output:
  x^T (pre-transposed on host as part of the sharding layout)
    Q^T[dq,t] = W_q[din,dq].T @ x^T[din,t]        (lhsT = W_q as stored)
    K^T[dk,t] = W_k[din,dk].T @ x^T[din,t]
    V[t,dv]   = x^T[din,t].T @ W_v[din,dv]        (natural layout)
  S^T[k,q] = K^T[dh,k].T @ Q^T[dh,q]              (keys on partitions)
  P^T = Exp(0.125*S^T + maskbias)  -- one fused ACT op per (head, ktile);
        maskbias varies along k = the partition dim, so it rides the
        per-partition bias operand. No max-subtraction: scores are
        N(0,1)-scaled so exp never overflows fp32.
  PV with P^T STATIONARY (full 128-deep contraction, 65-cycle matmuls):
    att_q[q, 0:64|Z] = P^T[k,q].T @ [V_h | 1][k, 65], accumulated over kt.
    This is ~2.2x cheaper on PE than the [65, 512] orientation (which
    fills only 65 of 128 output partitions).
  normalize on DVE: zinv = 1/att_q[:, 64] (per-partition scalar), then
    att_sb = att_q[:, 0:64] * zinv broadcast along free (step-0 read).
  att^T via PE transpose ([128 q, 64] -> [64, 128] blocks into [din, t])
  y[t,dc] = att^T[din,t].T @ W_out[din,dc] + b_out (natural layout -> DMA)

PSUM budget (8 banks): st 2x2 (S^T tiles, also time-shared by the V and
out projections as [128,512] halves -- avoids any pool-close barrier),
att_q 1x2, transpose 1x1, kq-filler 1x1.

Scheduling (the PE is the globally binding engine at ~233us busy, the
ACT exp stream is second at ~153us, so every idle PE cycle is wall
time):
  - 24 junk matmuls on a zeroed tile warm the HAM clock gate (PE at
    1.2GHz until ~3.4us of sustained activity) inside the DMA-ramp
    shadow, so the V projection starts at 2.4GHz.
  - DMA order is bandwidth-critical: mask first (a late mask head-of-
    line-blocks V's bias-adds in the strict-FIFO DVE queue), then bv,
    wv, xk in 512-col chunks (~128KB per dma_start spreads the 16
    queues; bigger single DMAs serialize, smaller ones waste the
    descriptor sweet spot), then the prefetched t2=0 K/Q weight blocks,
    xq, and the late-needed bo/wout.
  - exp(kt) on ACT runs while PE does S^T(kt+1) then the 8 PV matmuls
    of kt; K^T/Q^T projection matmul groups are popped between kt steps
    as filler. The S^T stream runs one unit AHEAD across (head-pair,
    q-chunk) boundaries so exp(0) of the next unit never waits out the
    boundary.
  - the first NJP=6 j-tiles of the output projection are accumulated
    into an SBUF scratch (bias folded in) as late-phase filler -- they
    only need ATT[0..5], so they pop once head-pair 6 starts -- leaving
    a j=6,7-only serial tail; y rows 0:TQ/2 additionally drain during
    the last unit's kt loop. y is written bf16 (the fp32 write would
    double the final DMA drain; bf16 costs ~3e-4 of absmax_rel).
"""

import os
import sys
import types
from contextlib import ExitStack

import numpy as np
import ml_dtypes

import concourse.bass as bass
import concourse.tile as tile
import concourse.mybir as mybir
from concourse import bacc, bass_utils

P = 128
DH = 64

F32 = mybir.dt.float32
BF16 = mybir.dt.bfloat16
F32R = mybir.dt.float32r

# Full-problem dims (hardcoded per contract).
FULL_DIMS = dict(B=4, T=2048, D=1024, H=16)

DEFAULT_CFG = dict(
    dt_x=BF16,      # xT / xqT storage (dram + sbuf)
    dt_w=BF16,      # W_qkv streaming blocks
    dt_kv=BF16,     # K^T and V(aug) sbuf storage; must equal dt_p
    dt_q=BF16,      # Q^T sbuf storage
    dt_p=BF16,      # P^T (softmax numerator) sbuf storage
    dt_att=BF16,    # att^T and W_out storage
    use_f32r=True,  # bitcast fp32 matmul operands to float32r (4x faster)
)


def _np_dt(dt):
    return {F32: np.float32, BF16: ml_dtypes.bfloat16}[dt]


def _install_ntff_shim():
    """The agent image's antenv lacks axon_hooks; bass_utils needs it for
    trace=True under axon. Provide it from the boot module."""
    if "antenv.axon_hooks" in sys.modules:
        return
    try:
        from trn_agent_boot.trn_boot import _ntff_profile_via_ctypes
        hook = _ntff_profile_via_ctypes("/opt/axon/libaxon_pjrt.so")
    except Exception:
        hook = None
    mod = types.ModuleType("antenv.axon_hooks")
    mod.get_axon_ntff_profile_hook = lambda: hook
    mod.set_axon_ntff_profile_hook = lambda h: None
    sys.modules["antenv.axon_hooks"] = mod


def _chunks(total, sz):
    out, off = [], 0
    while off < total:
        c = min(sz, total - off)
        out.append((off, c))
        off += c
    return out


def build_nc(dims, cfg, NKC):
    """Build the per-core SPMD program for NKC compacted key tiles."""
    T, D, H = dims["T"], dims["D"], dims["H"]
    assert H * DH == D
    TQ = T // 2           # queries per core
    NDIN = D // P         # contraction tiles for the projections
    NHT = H // 2          # head pairs (2 heads of 64 per 128 partitions)
    TKC = NKC * P         # compacted key positions
    FBV = min(512, D)     # dv-block for V compute
    FBO = min(512, D)     # dc-block for out projection
    D3 = 3 * D

    dt_x, dt_w = cfg["dt_x"], cfg["dt_w"]
    dt_kv, dt_q, dt_p, dt_att = cfg["dt_kv"], cfg["dt_q"], cfg["dt_p"], cfg["dt_att"]
    assert dt_p == dt_kv, "PV matmul needs matching operand dtypes"

    # SBUF headroom fallback for near-unmasked inputs (rare: the mask is
    # Bernoulli(0.5), so NKC ~ T/256; these trims only cost a little overlap)
    big = NKC > 12
    wblk_bufs = 2 if big else 3
    wv_bufs = 1 if big else 2
    pt_bufs = 2 if big else 4
    ob_bufs = 3 if big else 6
    dt_bias = BF16  # bias magnitudes ~0.06; bf16 rounding is ~2e-4 absolute

    def mm(ap):
        if cfg["use_f32r"] and ap.dtype == F32:
            return ap.bitcast(F32R)
        return ap

    nc = bacc.Bacc("TRN2", target_bir_lowering=False, debug=False)

    xkT_d = nc.dram_tensor("xkT", [D, TKC], dt_x, kind="ExternalInput")
    xqT_d = nc.dram_tensor("xqT", [D, TQ], dt_x, kind="ExternalInput")
    wqkv_d = nc.dram_tensor("wqkv", [D, D3], dt_w, kind="ExternalInput")
    wout_d = nc.dram_tensor("wout", [D, D], dt_att, kind="ExternalInput")
    bq_d = nc.dram_tensor("bq", [P, NDIN], F32, kind="ExternalInput")
    bk_d = nc.dram_tensor("bk", [P, NDIN], F32, kind="ExternalInput")
    bv_d = nc.dram_tensor("bv", [P, D], dt_bias, kind="ExternalInput")
    bo_d = nc.dram_tensor("bo", [P, D], dt_bias, kind="ExternalInput")
    maskm_d = nc.dram_tensor("maskm", [P, NKC], F32, kind="ExternalInput")
    ident_d = nc.dram_tensor("ident", [P, P], dt_att, kind="ExternalInput")
    y_d = nc.dram_tensor("y", [TQ, D], BF16, kind="ExternalOutput")

    in_names = ["xkT", "xqT", "wqkv", "wout", "bq", "bk", "bv", "bo",
                "maskm", "ident"]

    # wqkv viewed as [p, din_tile, col] so one DMA grabs a column block
    # across all NDIN din tiles.
    wqkv_v = wqkv_d.ap().rearrange("(j p) n -> p j n", p=P)
    wout_v = wout_d.ap().rearrange("(j p) n -> p j n", p=P)

    EXP = mybir.ActivationFunctionType.Exp

    with tile.TileContext(nc) as tc, ExitStack() as stk:
        misc = stk.enter_context(tc.tile_pool(name="misc", bufs=1))
        pers = stk.enter_context(tc.tile_pool(name="pers", bufs=1))

        # --- small persistent tiles (no DMAs yet: the first ~10us of DMA
        # bandwidth is reserved for the V-projection critical path) --------
        bv_sb = misc.tile([P, D], dt_bias, tag="bv", name="bv_sb")
        mf_sb = misc.tile([P, NKC], F32, tag="mf", name="mf_sb")
        id_sb = misc.tile([P, P], dt_att, tag="ident", name="id_sb")
        # mask first: tiny, and anything DVE-dependent on it must never
        # head-of-line-block the V bias-adds in the strict-FIFO DVE queue
        nc.sync.dma_start(out=mf_sb, in_=maskm_d.ap())

        # --- persistent big tensors ----------------------------------------
        KT = [pers.tile([P, TKC], dt_kv, tag=f"KT{i}", name=f"KT{i}")
              for i in range(NDIN)]
        QT = [pers.tile([P, TQ], dt_q, tag=f"QT{i}", name=f"QT{i}")
              for i in range(NDIN)]
        VA = [pers.tile([P, H * (DH + 1)], dt_kv, tag=f"VA{i}", name=f"VA{i}")
              for i in range(NKC)]
        ATT = [pers.tile([P, TQ], dt_att, tag=f"ATT{i}", name=f"ATT{i}")
               for i in range(NDIN)]

        # ones columns of the augmented V
        for kt in range(NKC):
            va_v = VA[kt].rearrange("p (h c) -> p h c", c=DH + 1)
            nc.vector.memset(va_v[:, :, DH:DH + 1], 1.0)

        # ========== Phase 1+2: projections interleaved with attention ======
        # V is computed first (every PV needs all of it). The K^T/Q^T
        # projection matmul groups are then fed into the attention emission
        # as filler work: phase 2 is ACT(exp)-throughput-bound and the PE
        # queue is in-order, so projection MMs slotted between attention MMs
        # keep the PE busy (and the HAM clock-gate warm) while ACT catches
        # up. Head h needs K^T/Q^T tile h//2, so the filler queue is ordered
        # by head-pair and drained ahead of each head's first matmul.
        with tc.tile_pool(name="ph1", bufs=1) as ph1, \
             tc.tile_pool(name="wstr", bufs=1) as wstr, \
             tc.tile_pool(name="ph2", bufs=1) as ph2, \
             tc.tile_pool(name="wvp", bufs=1) as wvp, \
             tc.tile_pool(name="stps", bufs=1, space="PSUM") as stps, \
             tc.tile_pool(name="kqps", bufs=1, space="PSUM") as kqps:

            # st tiles: [128, 1024] f32 (2 banks, 2 bufs). Attention S^T
            # uses the full width; the V projection and out projection use
            # [128, 512] halves of the same tag so they pipeline into/out of
            # attention with no pool barrier and no extra banks.
            def st_tile(nm):
                return stps.tile([P, 1024], F32, tag="st", bufs=2, name=nm)

            # the first V matmul group needs wv(0) and the xk tiles in
            # j-order; split the wv DMA per din-tile so it spreads across
            # DMA queues instead of serializing ~1MB on one queue, and feed
            # the xk columns in ascending-size chunks so group (dv2=0,kt=0)
            # is gated on only ~1.25MB of traffic.
            hpb = FBV // DH  # heads per dv block
            if True:
                # PE warm-up in the DMA shadow: the HAM clock gate holds the
                # PE at 1.2GHz until ~3.4us of sustained activity; junk
                # matmuls on a zeroed tile (no DMA deps) warm it for free.
                warm = misc.tile([P, 512], BF16, tag="warm", name="warm")
                nc.vector.memset(warm, 0.0)
                for wi in range(24):
                    wps = stps.tile([P, 1024], F32, tag="st", bufs=2,
                                    name=f"warm{wi}")
                    nc.tensor.matmul(wps[:, 0:512], warm[:, 0:P], warm,
                                     start=True, stop=True)

                # bv first: only 256KB, and the V bias-adds must never wait
                # on it (a late bv head-of-line-blocks the st psum ring).
                nc.sync.dma_start(out=bv_sb, in_=bv_d.ap())

                wvs = []
                for dv2 in range(D // FBV):
                    wvs.append(wvp.tile([P, NDIN, FBV], dt_w, tag="wv",
                                        bufs=wv_bufs, name=f"wv{dv2}"))

                def wv_dma(dv2):
                    for j in range(NDIN):
                        nc.sync.dma_start(
                            out=wvs[dv2][:, j, :],
                            in_=wqkv_v[:, j, 2 * D + dv2 * FBV:
                                       2 * D + (dv2 + 1) * FBV])

                xks = [ph1.tile([P, TKC], dt_x, tag=f"xk{j}", name=f"xk{j}")
                       for j in range(NDIN)]

                def xk_dma(off, csz):
                    for j in range(NDIN):
                        nc.sync.dma_start(
                            out=xks[j][:, off:off + csz],
                            in_=xkT_d.ap()[j * P:(j + 1) * P, off:off + csz])

                wv_dma(0)
                for oc in _chunks(TKC, 512):
                    xk_dma(*oc)
                wv_dma(1)
                nc.sync.dma_start(out=id_sb, in_=ident_d.ap())
                # prefetch the first K/Q weight blocks so the t2=0 filler
                # groups never wait on their DMAs behind the xq stream
                wbk0_pre = wstr.tile([P, NDIN, P], dt_w, tag="wblk",
                                     bufs=wblk_bufs, name="wbk0")
                nc.sync.dma_start(out=wbk0_pre, in_=wqkv_v[:, :, D:D + P])
                wbq0_pre = wstr.tile([P, NDIN, P], dt_w, tag="wblk",
                                     bufs=wblk_bufs, name="wbq0")
                nc.sync.dma_start(out=wbq0_pre, in_=wqkv_v[:, :, 0:P])
                def v_group(dv2, kt, ps):
                    psh = ps[:, 0:FBV]
                    wv = wvs[dv2]
                    for j in range(NDIN):
                        nc.tensor.matmul(
                            psh, mm(xks[j][:, kt * P:(kt + 1) * P]),
                            mm(wv[:, j, :]),
                            start=(j == 0), stop=(j == NDIN - 1))
                    va_v = VA[kt].rearrange("p (h c) -> p h c", c=DH + 1)
                    nc.vector.tensor_add(
                        va_v[:, dv2 * hpb:(dv2 + 1) * hpb, 0:DH],
                        psh.rearrange("p (h c) -> p h c", c=DH),
                        bv_sb[:, dv2 * FBV:(dv2 + 1) * FBV]
                        .rearrange("p (h c) -> p h c", c=DH))

                for dv2 in range(D // FBV):
                    for kt in range(NKC):
                        v_group(dv2, kt, st_tile(f"vps{dv2}_{kt}"))

            # mask bias prep: emitted only now so these DVE ops sit BEHIND
            # the V bias-adds in the strict-FIFO DVE queue.
            m1_sb = misc.tile([P, NKC], F32, tag="m1", name="m1_sb")
            nc.vector.tensor_scalar_add(m1_sb, mf_sb, -1.0)
            maskadd = misc.tile([P, NKC], F32, tag="maskadd",
                                name="maskadd")
            nc.vector.tensor_scalar_mul(maskadd, m1_sb, 1000.0)

            # loads not needed until the K/Q fillers and the projection --
            # emitted after V so they don't delay the first V matmuls.
            bq_sb = misc.tile([P, NDIN], F32, tag="bq", name="bq_sb")
            nc.sync.dma_start(out=bq_sb, in_=bq_d.ap())
            bk_sb = misc.tile([P, NDIN], F32, tag="bk", name="bk_sb")
            nc.sync.dma_start(out=bk_sb, in_=bk_d.ap())
            xqs = []
            for j in range(NDIN):
                xq = ph1.tile([P, TQ], dt_x, tag=f"xq{j}", name=f"xq{j}")
                nc.sync.dma_start(out=xq, in_=xqT_d.ap()[j * P:(j + 1) * P, :])
                xqs.append(xq)
            bo_sb = misc.tile([P, D], dt_bias, tag="bo", name="bo_sb")
            nc.sync.dma_start(out=bo_sb, in_=bo_d.ap())
            wout_sb = []
            for j in range(NDIN):
                wo = ph2.tile([P, D], dt_att, tag=f"wo{j}", name=f"wo{j}")
                nc.sync.dma_start(out=wo, in_=wout_v[:, j, :])
                wout_sb.append(wo)

            # --- K^T / Q^T filler work queue, ordered by head-pair -----
            def kq_dma(col0, nm):
                wb = wstr.tile([P, NDIN, P], dt_w, tag="wblk",
                               bufs=wblk_bufs, name=nm)
                nc.sync.dma_start(
                    out=wb, in_=wqkv_v[:, :, col0:col0 + P])
                return wb

            def kq_group(wb, xs, dst, bias, off, csz, nm):
                ps = kqps.tile([P, 512], F32, tag="kps", bufs=1, name=nm)
                for j in range(NDIN):
                    nc.tensor.matmul(
                        ps[:, :csz], mm(wb[:, j, :]),
                        mm(xs[j][:, off:off + csz]),
                        start=(j == 0), stop=(j == NDIN - 1))
                nc.vector.tensor_scalar_add(
                    dst[:, off:off + csz], ps[:, :csz], bias)

            work = []  # (hp, closure)
            for t2 in range(NDIN):
                wbk_hold, wbq_hold = {}, {}
                if t2 == 0:
                    wbk_hold["wb"] = wbk0_pre
                    wbq_hold["wb"] = wbq0_pre

                def mk_dma(hold, col0, nm):
                    def run():
                        hold["wb"] = kq_dma(col0, nm)
                    return run

                def mk_grp(hold, xs, dst, bias, off, csz, nm):
                    def run():
                        kq_group(hold["wb"], xs, dst, bias, off, csz, nm)
                    return run

                if t2 > 0:
                    work.append((t2, mk_dma(wbk_hold, D + t2 * P,
                                            f"wbk{t2}")))
                for off, csz in _chunks(TKC, 512):
                    work.append((t2, mk_grp(
                        wbk_hold, xks, KT[t2], bk_sb[:, t2:t2 + 1],
                        off, csz, f"kps{t2}_{off}")))
                if t2 > 0:
                    work.append((t2, mk_dma(wbq_hold, t2 * P, f"wbq{t2}")))
                for off, csz in _chunks(TQ, 512):
                    work.append((t2, mk_grp(
                        wbq_hold, xqs, QT[t2], bq_sb[:, t2:t2 + 1],
                        off, csz, f"qps{t2}_{off}")))

            # Out-projection split: j=0..NJP-1 accumulate into an SBUF
            # scratch (with the output bias folded in) as filler work
            # during the LAST head pair -- exactly where the K/Q filler
            # queue runs dry and the PE otherwise idles on exp -- leaving
            # only j=NJP..7 for the serial tail.
            NJP = 0 if big else 6
            if NJP:
                yacc = ph2.tile([P, TQ // P, D], F32, tag="yacc",
                                name="yacc")
            part_work = []

            def mk_part(tb, dc):
                def run():
                    ps = kqps.tile([P, 512], F32, tag="kps", bufs=1,
                                   name=f"pp{tb}_{dc}")
                    for j in range(NJP):
                        nc.tensor.matmul(
                            ps[:, :FBO],
                            mm(ATT[j][:, tb * P:(tb + 1) * P]),
                            mm(wout_sb[j][:, dc * FBO:(dc + 1) * FBO]),
                            start=(j == 0), stop=(j == NJP - 1))
                    nc.vector.tensor_add(
                        yacc[:, tb, dc * FBO:(dc + 1) * FBO], ps[:, :FBO],
                        bo_sb[:, dc * FBO:(dc + 1) * FBO])
                return run

            if NJP:
                for tb in range(TQ // P):
                    for dc in range(D // FBO):
                        part_work.append(mk_part(tb, dc))

            widx = [0]
            pidx = [0]

            def drain_kq(hp_needed):
                while widx[0] < len(work) and \
                        work[widx[0]][0] <= hp_needed:
                    work[widx[0]][1]()
                    widx[0] += 1

            def pop_kq(n=1, parts=False):
                for _ in range(n):
                    if widx[0] < len(work):
                        work[widx[0]][1]()
                        widx[0] += 1
                    elif parts and pidx[0] < len(part_work):
                        part_work[pidx[0]]()
                        pidx[0] += 1

            # --- attention ---------------------------------------------
            # Head PAIRS share one [128, 2*512] score tile: head 0's
            # q-chunk in cols [0,512), head 1's in [512,1024) (separate
            # psum banks). One TQ-wide exp covers both (same per-partition
            # mask bias). PV runs with P^T stationary: per (s2, 128-wide
            # q subtile), out_q[128 q, 65] += P^T.T @ [V_h|1], full-128
            # contraction, 65-cycle matmuls, accumulated over kt.
            slot = [0]
            qhs = _chunks(TQ, 512)
            STW = 512
            NSUB = STW // P  # 128-wide q subtiles per q-chunk
            fin_pend = [None]  # deferred normalize+transpose closure

            units = [(hp, qi, off, qcsz)
                     for hp in range(NHT)
                     for qi, (off, qcsz) in enumerate(qhs)]

            def mk_st(_hp, _qi, _off, _qcsz):
                def st_mm(kt):
                    stt = st_tile(f"st{_hp}_{_qi}_{kt}")
                    for s2 in range(2):
                        b2 = s2 * DH
                        nc.tensor.matmul(
                            stt[:, s2 * STW:s2 * STW + _qcsz],
                            mm(KT[_hp][b2:b2 + DH,
                                       kt * P:(kt + 1) * P]),
                            mm(QT[_hp][b2:b2 + DH,
                                       _off:_off + _qcsz]),
                            start=True, stop=True)
                    return stt
                return st_mm

            def tail_tb(tb):
                ob = ph2.tile([P, D], BF16, tag="ob", bufs=ob_bufs,
                              name=f"ob{tb}")
                for dc in range(D // FBO):
                    ps = st_tile(f"ops{tb}_{dc}")
                    psh = ps[:, 0:FBO]
                    for j in range(NJP, NDIN):
                        nc.tensor.matmul(
                            psh,
                            mm(ATT[j][:, tb * P:(tb + 1) * P]),
                            mm(wout_sb[j][:, dc * FBO:(dc + 1) * FBO]),
                            start=(j == NJP), stop=(j == NDIN - 1))
                    if NJP:
                        nc.vector.tensor_add(
                            ob[:, dc * FBO:(dc + 1) * FBO], psh,
                            yacc[:, tb, dc * FBO:(dc + 1) * FBO])
                    else:
                        nc.vector.tensor_add(
                            ob[:, dc * FBO:(dc + 1) * FBO], psh,
                            bo_sb[:, dc * FBO:(dc + 1) * FBO])
                for dq in range(2):
                    nc.sync.dma_start(
                        out=y_d.ap()[tb * P:(tb + 1) * P,
                                     dq * (D // 2):(dq + 1) * (D // 2)],
                        in_=ob[:, dq * (D // 2):(dq + 1) * (D // 2)])

            stfns = [mk_st(*u) for u in units]
            st_ahead = [None]  # next unit's st(0), emitted one unit early
            tail_done = set()

            for ui, (hp, qi, off, qcsz) in enumerate(units):
                if qi == 0:
                    drain_kq(hp)
                nsub = (qcsz + P - 1) // P

                # att_q psum: per s2 one [128, nsub, 65] tile (1 bank)
                aqs = [stps.tile([P, NSUB, DH + 1], F32, tag="attq",
                                 bufs=2, name=f"aq{hp}_{qi}_{s2}")
                       for s2 in range(2)]
                if st_ahead[0] is not None:
                    stt = st_ahead[0]
                    st_ahead[0] = None
                else:
                    stt = stfns[ui](0)
                for kt in range(NKC):
                    pt = ph2.tile([P, 2 * STW], dt_p, tag="pt",
                                  bufs=pt_bufs,
                                  name=f"pt{hp}_{qi}_{kt}")
                    if qcsz == STW:
                        nc.scalar.activation(
                            out=pt, in_=stt, func=EXP,
                            bias=maskadd[:, kt:kt + 1], scale=0.125)
                    else:
                        for s2 in range(2):
                            nc.scalar.activation(
                                out=pt[:, s2 * STW:s2 * STW + qcsz],
                                in_=stt[:, s2 * STW:s2 * STW + qcsz],
                                func=EXP,
                                bias=maskadd[:, kt:kt + 1],
                                scale=0.125)
                    if kt + 1 < NKC:
                        stt = stfns[ui](kt + 1)
                    elif ui + 1 < len(units):
                        # S-stream runs one unit AHEAD: the next unit's
                        # st(0) is emitted before this unit's last PVs so
                        # its exp(0) never waits out the unit boundary.
                        if units[ui + 1][1] == 0:
                            drain_kq(units[ui + 1][0])
                        st_ahead[0] = stfns[ui + 1](0)
                    # previous unit's finalize lands here: its PE
                    # transposes slot into the wait for exp(0).
                    if kt == 0 and fin_pend[0] is not None:
                        fin_pend[0]()
                        fin_pend[0] = None
                    # start=True only on the tile's FIRST write: the
                    # hardware zero-region is the whole 2KB bank, so a
                    # per-qsub start would wipe earlier qsubs' kt=0
                    # results. Later qsubs' first writes land on
                    # still-pending bytes and overwrite correctly.
                    for s2 in range(2):
                        h2 = 2 * hp + s2
                        for sq in range(nsub):
                            scs = min(P, qcsz - sq * P)
                            nc.tensor.matmul(
                                aqs[s2][0:scs, sq, :],
                                mm(pt[:, s2 * STW + sq * P:
                                      s2 * STW + sq * P + scs]),
                                mm(VA[kt][:, h2 * (DH + 1):
                                          (h2 + 1) * (DH + 1)]),
                                start=(kt == 0 and sq == 0),
                                stop=(kt == NKC - 1 and sq == nsub - 1),
                                skip_group_check=True)
                    slot[0] += 1
                    # partials read ATT[0..NJP-1]; ATT[NJP-1] is written by
                    # fin(NJP-1, qi=1), emitted at unit (NJP, 0) kt==0 -- so
                    # partial pops are safe only from (NJP, 0) kt>=1 on.
                    psafe = ui > 2 * NJP or (ui == 2 * NJP and kt >= 1)
                    if slot[0] % 3 == 0:
                        pop_kq(1, parts=psafe)
                    # during the last unit, rows 0:TQ/2 of y depend only on
                    # fin(last hp, qi=0) and the finished partials -- emit
                    # their tail groups now so their DMAs drain early.
                    if (ui == len(units) - 1 and kt >= 1
                            and pidx[0] >= len(part_work)
                            and widx[0] >= len(work)
                            and len(tail_done) < TQ // (2 * P)):
                        tb = len(tail_done)
                        tail_tb(tb)
                        tail_done.add(tb)

                # Deferred finalize: 1/Z on DVE (Z = ones-column 64 of
                # att_q; per-partition scalars in this orientation),
                # normalize+downcast with a step-0 free-dim broadcast
                # read of zinv, then PE-transpose the [128 q, 64]
                # blocks into ATT's [din, t] layout. Runs after the
                # next unit's first exp is queued so the PE never
                # stalls on the DVE chain.
                def fin(_hp=hp, _qi=qi, _off=off, _qcsz=qcsz,
                        _aqs=aqs, _nsub=nsub):
                    zinv = ph2.tile([P, 2, NSUB], F32, tag="zinv",
                                    bufs=2, name=f"zi{_hp}_{_qi}")
                    asb = ph2.tile([P, NSUB, 2, DH], dt_att, tag="asb",
                                   bufs=2, name=f"as{_hp}_{_qi}")
                    tp = stps.tile([P, STW], dt_att, tag="tp",
                                   bufs=1, name=f"tp{_hp}_{_qi}")
                    for s2 in range(2):
                        nc.vector.reciprocal(
                            zinv[:, s2, 0:_nsub],
                            _aqs[s2][:, 0:_nsub, DH])
                        zbc = bass.AP(
                            tensor=zinv.tensor,
                            offset=zinv.offset + s2 * NSUB,
                            ap=[zinv.ap[0], [1, _nsub], [0, DH]])
                        nc.vector.tensor_mul(
                            asb[:, 0:_nsub, s2, :],
                            _aqs[s2][:, 0:_nsub, 0:DH], zbc)
                    # one transpose per q-subtile covers BOTH heads: the
                    # strided lhsT [128 q, (2 s2, 64 dh)] transposes to
                    # [128 din, q] -- exactly ATT's layout.
                    for sq in range(_nsub):
                        scs = min(P, _qcsz - sq * P)
                        nc.tensor.transpose(
                            tp[:, sq * P:sq * P + scs],
                            asb[0:scs, sq, :, :],
                            id_sb[0:scs, 0:scs])
                    nc.vector.tensor_copy(
                        ATT[_hp][:, _off:_off + _qcsz],
                        tp[:, 0:_qcsz])
                fin_pend[0] = fin
            fin_pend[0]()
            fin_pend[0] = None
            pop_kq(len(work))

            # ========= Phase 3: output projection tail =================
            # Only j=NJP..7 remain (the rest was absorbed as phase-2
            # filler); rides the st psum tag so no pool-close barrier
            # separates attention from the projection. One full-row ob
            # tile per tb gives the y DMA 2KB-per-partition descriptors.
            while pidx[0] < len(part_work):
                part_work[pidx[0]]()
                pidx[0] += 1
            for tb in range(TQ // P):
                if tb not in tail_done:
                    tail_tb(tb)

    nc.compile()
    return nc, in_names


def shard_inputs(dims, cfg, NKC, x, mask, W_qkv, b_qkv, W_out, b_out):
    """Host-side sharding: slices, layout transposes/permutation, bias
    tiling. The key permutation puts unmasked keys first (padding keeps
    mask=0 so the device-side bias kills it)."""
    B, T, D = dims["B"], dims["T"], dims["D"]
    TQ = T // 2
    NDIN = D // P
    TKC = NKC * P
    npx = _np_dt(cfg["dt_x"])
    npw = _np_dt(cfg["dt_w"])
    npa = _np_dt(cfg["dt_att"])

    x = np.asarray(x)
    mask = np.asarray(mask)
    W_qkv = np.asarray(W_qkv)
    b_qkv = np.asarray(b_qkv)
    W_out = np.asarray(W_out)
    b_out = np.asarray(b_out)

    wqkv_c = np.ascontiguousarray(W_qkv.astype(npw))
    wout_c = np.ascontiguousarray(W_out.astype(npa))
    bq = np.ascontiguousarray(b_qkv[:D].reshape(NDIN, P).T.astype(np.float32))
    bk = np.ascontiguousarray(
        b_qkv[D:2 * D].reshape(NDIN, P).T.astype(np.float32))
    np_bias = _np_dt(BF16)
    bv = np.ascontiguousarray(
        np.broadcast_to(b_qkv[2 * D:], (P, D)).astype(np_bias))
    bo = np.ascontiguousarray(
        np.broadcast_to(b_out, (P, D)).astype(np_bias))
    ident = np.ascontiguousarray(np.eye(P, dtype=npa))

    in_maps = []
    percore = {}
    for b in range(B):
        mb = mask[b, 0, 0]
        idx_on = np.nonzero(mb == 1)[0]
        perm = np.zeros(TKC, dtype=np.int64)  # pad with key 0 (masked off)
        perm[:len(idx_on)] = idx_on
        mc = np.zeros(TKC, dtype=np.float32)
        mc[:len(idx_on)] = 1.0
        xkT = np.ascontiguousarray(x[b][perm].T.astype(npx))
        maskm = np.ascontiguousarray(mc.reshape(NKC, P).T)
        percore[b] = (xkT, maskm)

    for c in range(2 * B):
        b, hf = c // 2, c % 2
        xkT, maskm = percore[b]
        xqT = np.ascontiguousarray(
            x[b, hf * TQ:(hf + 1) * TQ, :].T.astype(npx))
        in_maps.append(dict(
            xkT=xkT, xqT=xqT, wqkv=wqkv_c, wout=wout_c,
            bq=bq, bk=bk, bv=bv, bo=bo, maskm=maskm, ident=ident))
    return in_maps


_CACHE = {}
LAST_EXEC_NS = None


def kernel(x, mask, W_qkv, b_qkv, W_out, b_out):
    global LAST_EXEC_NS
    dims = FULL_DIMS
    cfg = DEFAULT_CFG
    _install_ntff_shim()

    mask = np.asarray(mask)
    counts = mask.reshape(dims["B"], -1).sum(1)
    NKC = max(1, int(np.ceil(counts.max() / P)))
    NKC = min(NKC, dims["T"] // P)

    if NKC not in _CACHE:
        _CACHE[NKC] = build_nc(dims, cfg, NKC)
    nc, _ = _CACHE[NKC]

    in_maps = shard_inputs(dims, cfg, NKC, x, mask, W_qkv, b_qkv,
                           W_out, b_out)
    trace = bool(os.environ.get("KERNEL_TRACE"))
    res = bass_utils.run_bass_kernel_spmd(
        nc, in_maps, core_ids=list(range(8)), trace=trace,
        tmpdir=os.environ.get("KERNEL_TRACE_DIR") or None)
    LAST_EXEC_NS = res.exec_time_ns

    B, T, D = dims["B"], dims["T"], dims["D"]
    TQ = T // 2
    out = np.empty((B, T, D), dtype=np.float32)
    for c in range(2 * B):
        b, hf = c // 2, c % 2
        out[b, hf * TQ:(hf + 1) * TQ, :] = np.asarray(
            res.results[c]["y"], dtype=np.float32)
    return out
